# revision 30
# baseline (speedup 1.0000x reference)
"""Causal self-attention (b=4, t=2048, c=1024, h=16, d=64) on 8 TRN2 cores.

Sharding: core i -> batch i//2, head-half i%2 (8 heads), out-col-half i%2.
Per core: QKV (bf16) for its heads over its batch, flash-style causal
attention in transposed layout, pairwise AllGather of y^T, half of the
output projection columns. Output is exact-shape f32.
"""
import numpy as np
import ml_dtypes

import concourse.bass as bass
import concourse.mybir as mybir
import concourse.tile as tile
from concourse.tile_rust import add_dep_helper
from concourse import bacc
from concourse.bass_utils import run_bass_kernel_spmd

BF16 = mybir.dt.bfloat16
F32 = mybir.dt.float32

B, T, C = 4, 2048, 1024
H, D = 16, 64
HL = 8            # heads per core (local)
NP = HL // 2      # head pairs per core
QB = 512          # q block (free dim)
NQB = T // QB     # 4 q blocks
NKT = T // 128    # 16 kv tiles
NCH = C // 128    # 8 contraction chunks
G = 1             # kv tiles per score/exp group

_NC_CACHE = {}


def build_nc():
    if "nc" in _NC_CACHE:
        return _NC_CACHE["nc"]
    nc = bacc.Bacc("TRN2", target_bir_lowering=False, debug=False, num_devices=8)
    xT = nc.dram_tensor("xT", [C, T], BF16, kind="ExternalInput")
    wk = nc.dram_tensor("wk", [C, 512], BF16, kind="ExternalInput")
    wq = nc.dram_tensor("wq", [C, 512], BF16, kind="ExternalInput")
    wv = nc.dram_tensor("wv", [C, 512], BF16, kind="ExternalInput")
    wp = nc.dram_tensor("wp", [C, 512], BF16, kind="ExternalInput")
    msk = nc.dram_tensor("msk", [128, 4 * QB], BF16, kind="ExternalInput")
    out = nc.dram_tensor("out", [T, 512], F32, kind="ExternalOutput")

    with tile.TileContext(nc) as tc:
        with tc.tile_pool(name="w", bufs=1) as wpool, \
             tc.tile_pool(name="x", bufs=1) as xpool, \
             tc.tile_pool(name="kqv", bufs=1) as kqv, \
             tc.tile_pool(name="att", bufs=2) as att, \
             tc.tile_pool(name="y", bufs=1) as ypool, \
             tc.tile_pool(name="ps", bufs=1, space="PSUM") as ps, \
             tc.tile_pool(name="dram", bufs=1, space="DRAM") as dram:

            # ---- load inputs ----
            xT_sb = [xpool.tile([128, T], BF16, name=f"xT{ch}") for ch in range(NCH)]
            wk_sb = [wpool.tile([128, 512], BF16, name=f"wk{ch}") for ch in range(NCH)]
            wq_sb = [wpool.tile([128, 512], BF16, name=f"wq{ch}") for ch in range(NCH)]
            wv_sb = [wpool.tile([128, 512], BF16, name=f"wv{ch}") for ch in range(NCH)]
            wp_sb = [wpool.tile([128, 512], BF16, name=f"wp{ch}") for ch in range(NCH)]
            for ch in range(NCH):
                sl = slice(128 * ch, 128 * (ch + 1))
                eng = nc.sync if ch % 2 == 0 else nc.scalar
                other = nc.scalar if ch % 2 == 0 else nc.sync
                eng.dma_start(out=wk_sb[ch][:], in_=wk.ap()[sl, :])
                eng.dma_start(out=xT_sb[ch][:], in_=xT.ap()[sl, :])
                other.dma_start(out=wv_sb[ch][:], in_=wv.ap()[sl, :])
                other.dma_start(out=wq_sb[ch][:], in_=wq.ap()[sl, :])
            for ch in range(NCH):
                nc.scalar.dma_start(out=wp_sb[ch][:], in_=wp.ap()[128 * ch:128 * (ch + 1), :])
            msk_sb = wpool.tile([128, 4 * QB], BF16)
            nc.scalar.dma_start(out=msk_sb[:], in_=msk.ap()[:])

            # ---- V (token-major, with ones column per head) ----
            v_sb = [kqv.tile([128, HL * 65], BF16, name=f"v{tt}") for tt in range(NKT)]

            def v_group(tt):
                v3 = v_sb[tt].rearrange("p (g e) -> p g e", g=HL)
                v_ps = ps.tile([128, 512], F32, tag="qkv", bufs=2, name="vps")
                for ch in range(NCH):
                    nc.tensor.matmul(
                        v_ps[:],
                        xT_sb[ch][:, 128 * tt:128 * (tt + 1)],
                        wv_sb[ch][:],
                        start=(ch == 0), stop=(ch == NCH - 1),
                    )
                nc.vector.tensor_copy(
                    out=v3[:, :, 0:64],
                    in_=v_ps[:].rearrange("p (g e) -> p g e", g=HL),
                )
                nc.vector.memset(v3[:, :, 64:65], 1.0)

            # ---- K^T, Q^T (head-pair tiles [128, T]) ----
            kT_sb = [kqv.tile([128, T], BF16, name=f"kT{p}") for p in range(NP)]
            qT_sb = [kqv.tile([128, T], BF16, name=f"qT{p}") for p in range(NP)]
            yT_sb = [ypool.tile([128, T], BF16, name=f"yT{p}", tag="yt", bufs=NP) for p in range(NP)]
            sums_pack = ypool.tile([128, 128], F32)
            recip_pack = ypool.tile([128, 128], F32)
            recip_pbf = ypool.tile([128, 128], BF16)

            rdram = dram.tile([HL, T], BF16)
            cc_in = [dram.tile([128, T], BF16, name=f"ccin{p}") for p in range(NP)]
            cc_out = [dram.tile([256, T], BF16, name=f"ccout{p}") for p in range(NP)]
            yg_sb = [ypool.tile([128, T], BF16, name=f"yg{j}", tag="yg", bufs=2 * NP) for j in range(2 * NP)]

            def kq_group(p, w_sb, dst, nb, evac_eng=None):
                kq_ps = ps.tile([128, 512], F32, tag="qkv", bufs=2, name="kqps")
                for ch in range(NCH):
                    nc.tensor.matmul(
                        kq_ps[:],
                        w_sb[ch][:, 128 * p:128 * (p + 1)],
                        xT_sb[ch][:, 512 * nb:512 * (nb + 1)],
                        start=(ch == 0), stop=(ch == NCH - 1),
                    )
                if evac_eng == "scalar":
                    nc.scalar.copy(out=dst[p][:, 512 * nb:512 * (nb + 1)], in_=kq_ps[:])
                else:
                    nc.vector.tensor_copy(out=dst[p][:, 512 * nb:512 * (nb + 1)], in_=kq_ps[:])

            def kq_thunks(p, evac_eng=None):
                return [
                    (lambda w_sb=w_sb, dst=dst, nb=nb: kq_group(p, w_sb, dst, nb, evac_eng))
                    for (w_sb, dst) in ((wk_sb, kT_sb), (wq_sb, qT_sb))
                    for nb in range(T // 512)
                ]

            att_marker = {}
            yg_dmas = []
            for t in kq_thunks(0, evac_eng="scalar"):
                t()
            for tt in range(4):
                v_group(tt)
            for p in range(NP):
                pend = kq_thunks(p + 1) if p + 1 < NP else []
                if p == 0:
                    pend = [(lambda tt=tt: v_group(tt)) for tt in range(4, NKT)] + pend
                # ---- attention for this head pair ----
                for qb in range(NQB):
                    nkv = 4 * (qb + 1)   # kv tiles for this q block
                    y_ps = [ps.tile([128, 512], F32, tag=f"y{h}", bufs=1, name=f"yps{h}") for h in range(2)]
                    for g0 in range(0, nkv, G):
                        s_ps = ps.tile([128, 2 * G * 512], F32, tag="s", bufs=2)
                        for h in range(2):
                            hsl = slice(64 * h, 64 * (h + 1))
                            for j in range(G):
                                kt = g0 + j
                                mm = nc.tensor.matmul(
                                    s_ps[:, 512 * (G * h + j):512 * (G * h + j + 1)],
                                    kT_sb[p][hsl, 128 * kt:128 * (kt + 1)],
                                    qT_sb[p][hsl, QB * qb:QB * (qb + 1)],
                                    start=True, stop=True,
                                )
                                att_marker.setdefault(p, mm)
                                att_last = mm
                        p_sb = att.tile([128, 2 * G * 512], BF16, tag="p", bufs=4)
                        nc.scalar.activation(
                            out=p_sb[:],
                            in_=s_ps[:],
                            func=mybir.ActivationFunctionType.Exp,
                            scale=float(D) ** -0.5,
                        )
                        for h in range(2):
                            for j in range(G):
                                kt = g0 + j
                                r = kt - 4 * qb
                                if r >= 0:  # diagonal region: apply causal mask
                                    nc.vector.tensor_mul(
                                        out=p_sb[:, 512 * (G * h + j):512 * (G * h + j + 1)],
                                        in0=p_sb[:, 512 * (G * h + j):512 * (G * h + j + 1)],
                                        in1=msk_sb[:, QB * r:QB * (r + 1)],
                                    )
                        for h in range(2):
                            for j in range(G):
                                kt = g0 + j
                                v3 = v_sb[kt].rearrange("p (g e) -> p g e", g=HL)
                                nc.tensor.matmul(
                                    y_ps[h][0:65, :],
                                    v3[:, 2 * p + h, :],
                                    p_sb[:, 512 * (G * h + j):512 * (G * h + j + 1)],
                                    start=(kt == 0), stop=(kt == nkv - 1),
                                )
                        if pend and (p == 0 or qb >= 2):
                            if p == 0 or g0 % 2 == 1 or len(pend) > (NQB - qb) * 4:
                                pend.pop(0)()
                    # evacuate y (unnormalized) + packed sums
                    qsl = slice(QB * qb, QB * (qb + 1))
                    nc.vector.tensor_copy(out=yT_sb[p][0:64, qsl], in_=y_ps[0][0:64, :])
                    ytmp = att.tile([64, 512], BF16, tag="ytmp", bufs=2)
                    nc.vector.tensor_copy(out=ytmp[:], in_=y_ps[1][0:64, :])
                    nc.sync.dma_start(out=yT_sb[p][64:128, qsl], in_=ytmp[:])
                    for h in range(2):
                        stg = att.tile([65, 512], F32, tag=f"sumstg{h}", bufs=2, name=f"stg{h}")
                        nc.vector.tensor_copy(out=stg[64:65, :], in_=y_ps[h][64:65, :])
                        u = 8 * p + 2 * qb + h
                        nc.sync.dma_start(out=sums_pack[4 * u:4 * (u + 1), :], in_=stg[64:65, :])
                    # per-qb normalize (recip over the pair's 32 rows is idempotent)
                    prows = slice(32 * p, 32 * (p + 1))
                    nc.vector.reciprocal(out=recip_pack[prows, :], in_=sums_pack[prows, :])
                    nc.vector.tensor_copy(out=recip_pbf[prows, :], in_=recip_pack[prows, :])
                    for h in range(2):
                        base = 32 * p + 8 * qb + 4 * h
                        nc.sync.dma_start(
                            out=rdram[2 * p + h:2 * p + h + 1, qsl],
                            in_=recip_pbf[base:base + 4, :],
                        )
                    rb = att.tile([128, QB], BF16, tag="rb", bufs=2)
                    nc.sync.dma_start(out=rb[0:64, :], in_=rdram[2 * p:2 * p + 1, qsl].to_broadcast([64, QB]))
                    nc.sync.dma_start(out=rb[64:128, :], in_=rdram[2 * p + 1:2 * p + 2, qsl].to_broadcast([64, QB]))
                    nc.gpsimd.tensor_mul(out=yT_sb[p][:, qsl], in0=yT_sb[p][:, qsl], in1=rb[:])

                while pend:
                    pend.pop(0)()
                nc.sync.dma_start(out=cc_in[p][:], in_=yT_sb[p][:])
                nc.gpsimd.collective_compute(
                    "AllGather",
                    mybir.AluOpType.bypass,
                    replica_groups=[[0, 1], [2, 3], [4, 5], [6, 7]],
                    ins=[cc_in[p][:].opt()],
                    outs=[cc_out[p][:].opt()],
                )
            for p in range(NP):
                yg_dmas.append(nc.gpsimd.dma_start(out=yg_sb[p][:], in_=cc_out[p][0:128, :]))
                yg_dmas.append(nc.gpsimd.dma_start(out=yg_sb[NP + p][:], in_=cc_out[p][128:256, :]))
            for dma in yg_dmas:
                add_dep_helper(dma.ins, att_marker[NP - 1].ins, sync=True,
                               reason="defer yg readback past attention start of last pair")
            # ---- projection pass A: pairs 0..2 (runs as filler inside last pair's attention) ----
            o_part = [ypool.tile([128, 512], BF16, name=f"opart{tt}", tag="opart", bufs=NKT)
                      for tt in range(NKT)]
            chunksA = [(r, p) for r in range(2) for p in range(NP - 1)]

            def projA(tt):
                o_ps = ps.tile([128, 512], F32, tag="qkv", bufs=2, name="ops")
                first = None
                for ci, (r, p) in enumerate(chunksA):
                    mm = nc.tensor.matmul(
                        o_ps[:],
                        yg_sb[r * NP + p][:, 128 * tt:128 * (tt + 1)],
                        wp_sb[4 * r + p][:],
                        start=(ci == 0), stop=(ci == len(chunksA) - 1),
                    )
                    first = first or mm
                nc.vector.tensor_copy(out=o_part[tt][:], in_=o_ps[:])
                return first

            for tt in range(NKT - 4):
                projA(tt)
            for tt in range(NKT - 4, NKT):
                first_mm = projA(tt)
                add_dep_helper(first_mm.ins, att_last.ins, sync=True,
                               reason="hold A2 groups for the last-AG window")
            # ---- projection pass B: pair 3 chunks + partial, after last AG ----
            for tt in range(NKT):
                o_ps2 = ps.tile([128, 512], F32, tag="qkv", bufs=2)
                for ri, r in enumerate(range(2)):
                    nc.tensor.matmul(
                        o_ps2[:],
                        yg_sb[r * NP + NP - 1][:, 128 * tt:128 * (tt + 1)],
                        wp_sb[4 * r + NP - 1][:],
                        start=(ri == 0), stop=(ri == 1),
                    )
                o_sb = att.tile([128, 512], F32, tag="osb", bufs=3)
                nc.vector.scalar_tensor_tensor(
                    out=o_sb[:], in0=o_ps2[:], scalar=1.0, in1=o_part[tt][:],
                    op0=mybir.AluOpType.mult, op1=mybir.AluOpType.add,
                )
                eng = nc.sync if tt % 2 == 0 else nc.scalar
                eng.dma_start(out=out.ap()[128 * tt:128 * (tt + 1), :], in_=o_sb[:])

    nc.compile()
    _NC_CACHE["nc"] = nc
    return nc


def make_in_maps(x, w_qkv, w_proj):
    bf = ml_dtypes.bfloat16
    # causal staircase masks for the 4 diagonal kv-tile offsets
    i = np.arange(128)[:, None]
    j = np.arange(QB)[None, :]
    msk = np.concatenate(
        [(r * 128 + i <= j).astype(bf) for r in range(4)], axis=1
    )  # [128, 2048]
    in_maps = []
    for core in range(8):
        beta, eta = core // 2, core % 2
        xT = np.ascontiguousarray(x[beta].T).astype(bf)
        wk = w_qkv[:, C + 512 * eta: C + 512 * (eta + 1)].astype(bf)
        wq = w_qkv[:, 512 * eta: 512 * (eta + 1)].astype(bf)
        wv = w_qkv[:, 2 * C + 512 * eta: 2 * C + 512 * (eta + 1)].astype(bf)
        wp = w_proj[:, 512 * eta: 512 * (eta + 1)].astype(bf)
        in_maps.append({"xT": xT, "wk": np.ascontiguousarray(wk),
                        "wq": np.ascontiguousarray(wq),
                        "wv": np.ascontiguousarray(wv),
                        "wp": np.ascontiguousarray(wp), "msk": msk})
    return in_maps


def assemble(results):
    out = np.empty((B, T, C), np.float32)
    for core in range(8):
        beta, eta = core // 2, core % 2
        out[beta, :, 512 * eta: 512 * (eta + 1)] = results[core]["out"]
    return out


def kernel(x, w_qkv, w_proj):
    x = np.asarray(x, np.float32)
    w_qkv = np.asarray(w_qkv, np.float32)
    w_proj = np.asarray(w_proj, np.float32)
    nc = build_nc()
    in_maps = make_in_maps(x, w_qkv, w_proj)
    res = run_bass_kernel_spmd(nc, in_maps, core_ids=list(range(8)))
    return assemble(res.results)


# revision 31
# speedup vs baseline: 1.0058x; 1.0058x over previous
"""Causal self-attention (b=4, t=2048, c=1024, h=16, d=64) on 8 TRN2 cores.

Sharding: core i -> batch i//2, head-half i%2 (8 heads), out-col-half i%2.
Per core: QKV (bf16) for its heads over its batch, flash-style causal
attention in transposed layout, pairwise AllGather of y^T, half of the
output projection columns. Output is exact-shape f32.
"""
import numpy as np
import ml_dtypes

import concourse.bass as bass
import concourse.mybir as mybir
import concourse.tile as tile
from concourse.tile_rust import add_dep_helper
from concourse import bacc
from concourse.bass_utils import run_bass_kernel_spmd

BF16 = mybir.dt.bfloat16
F32 = mybir.dt.float32

B, T, C = 4, 2048, 1024
H, D = 16, 64
HL = 8            # heads per core (local)
NP = HL // 2      # head pairs per core
QB = 512          # q block (free dim)
NQB = T // QB     # 4 q blocks
NKT = T // 128    # 16 kv tiles
NCH = C // 128    # 8 contraction chunks
G = 1             # kv tiles per score/exp group

_NC_CACHE = {}


def build_nc():
    if "nc" in _NC_CACHE:
        return _NC_CACHE["nc"]
    nc = bacc.Bacc("TRN2", target_bir_lowering=False, debug=False, num_devices=8)
    xT = nc.dram_tensor("xT", [C, T], BF16, kind="ExternalInput")
    wk = nc.dram_tensor("wk", [C, 512], BF16, kind="ExternalInput")
    wq = nc.dram_tensor("wq", [C, 512], BF16, kind="ExternalInput")
    wv = nc.dram_tensor("wv", [C, 512], BF16, kind="ExternalInput")
    wp = nc.dram_tensor("wp", [C, 512], BF16, kind="ExternalInput")
    msk = nc.dram_tensor("msk", [128, 4 * QB], BF16, kind="ExternalInput")
    out = nc.dram_tensor("out", [T, 512], F32, kind="ExternalOutput")

    with tile.TileContext(nc) as tc:
        with tc.tile_pool(name="w", bufs=1) as wpool, \
             tc.tile_pool(name="x", bufs=1) as xpool, \
             tc.tile_pool(name="kqv", bufs=1) as kqv, \
             tc.tile_pool(name="att", bufs=2) as att, \
             tc.tile_pool(name="y", bufs=1) as ypool, \
             tc.tile_pool(name="ps", bufs=1, space="PSUM") as ps, \
             tc.tile_pool(name="dram", bufs=1, space="DRAM") as dram:

            # ---- load inputs ----
            xT_sb = [xpool.tile([128, T], BF16, name=f"xT{ch}") for ch in range(NCH)]
            wk_sb = [wpool.tile([128, 512], BF16, name=f"wk{ch}") for ch in range(NCH)]
            wq_sb = [wpool.tile([128, 512], BF16, name=f"wq{ch}") for ch in range(NCH)]
            wv_sb = [wpool.tile([128, 512], BF16, name=f"wv{ch}") for ch in range(NCH)]
            wp_sb = [wpool.tile([128, 512], BF16, name=f"wp{ch}") for ch in range(NCH)]
            for ch in range(NCH):
                sl = slice(128 * ch, 128 * (ch + 1))
                eng = nc.sync if ch % 2 == 0 else nc.scalar
                other = nc.scalar if ch % 2 == 0 else nc.sync
                eng.dma_start(out=wk_sb[ch][:], in_=wk.ap()[sl, :])
                eng.dma_start(out=xT_sb[ch][:], in_=xT.ap()[sl, :])
                other.dma_start(out=wv_sb[ch][:], in_=wv.ap()[sl, :])
                other.dma_start(out=wq_sb[ch][:], in_=wq.ap()[sl, :])
            for ch in range(NCH):
                nc.scalar.dma_start(out=wp_sb[ch][:], in_=wp.ap()[128 * ch:128 * (ch + 1), :])
            msk_sb = wpool.tile([128, 4 * QB], BF16)
            nc.scalar.dma_start(out=msk_sb[:], in_=msk.ap()[:])

            # ---- V (token-major, with ones column per head) ----
            v_sb = [kqv.tile([128, HL * 65], BF16, name=f"v{tt}") for tt in range(NKT)]

            def v_group(tt):
                v3 = v_sb[tt].rearrange("p (g e) -> p g e", g=HL)
                v_ps = ps.tile([128, 512], F32, tag="qkv", bufs=2, name="vps")
                for ch in range(NCH):
                    nc.tensor.matmul(
                        v_ps[:],
                        xT_sb[ch][:, 128 * tt:128 * (tt + 1)],
                        wv_sb[ch][:],
                        start=(ch == 0), stop=(ch == NCH - 1),
                    )
                nc.vector.tensor_copy(
                    out=v3[:, :, 0:64],
                    in_=v_ps[:].rearrange("p (g e) -> p g e", g=HL),
                )
                nc.vector.memset(v3[:, :, 64:65], 1.0)

            # ---- K^T, Q^T (head-pair tiles [128, T]) ----
            kT_sb = [kqv.tile([128, T], BF16, name=f"kT{p}") for p in range(NP)]
            qT_sb = [kqv.tile([128, T], BF16, name=f"qT{p}") for p in range(NP)]
            yT_sb = [ypool.tile([128, T], BF16, name=f"yT{p}", tag="yt", bufs=NP) for p in range(NP)]
            sums_pack = ypool.tile([128, 128], F32)
            recip_pack = ypool.tile([128, 128], F32)
            recip_pbf = ypool.tile([128, 128], BF16)

            rdram = dram.tile([HL, T], BF16)
            cc_in = [dram.tile([128, T], BF16, name=f"ccin{p}") for p in range(NP)]
            cc_out = [dram.tile([256, T], BF16, name=f"ccout{p}") for p in range(NP)]
            yg_sb = [ypool.tile([128, T], BF16, name=f"yg{j}", tag="yg", bufs=2 * NP) for j in range(2 * NP)]

            def kq_group(p, w_sb, dst, nb, evac_eng=None):
                kq_ps = ps.tile([128, 512], F32, tag="qkv", bufs=2, name="kqps")
                for ch in range(NCH):
                    nc.tensor.matmul(
                        kq_ps[:],
                        w_sb[ch][:, 128 * p:128 * (p + 1)],
                        xT_sb[ch][:, 512 * nb:512 * (nb + 1)],
                        start=(ch == 0), stop=(ch == NCH - 1),
                    )
                if evac_eng == "scalar":
                    nc.scalar.copy(out=dst[p][:, 512 * nb:512 * (nb + 1)], in_=kq_ps[:])
                else:
                    nc.vector.tensor_copy(out=dst[p][:, 512 * nb:512 * (nb + 1)], in_=kq_ps[:])

            def kq_thunks(p, evac_eng=None):
                return [
                    (lambda w_sb=w_sb, dst=dst, nb=nb: kq_group(p, w_sb, dst, nb, evac_eng))
                    for (w_sb, dst) in ((wk_sb, kT_sb), (wq_sb, qT_sb))
                    for nb in range(T // 512)
                ]

            att_marker = {}
            yg_dmas = []
            for t in kq_thunks(0, evac_eng="scalar"):
                t()
            for tt in range(4):
                v_group(tt)
            for p in range(NP):
                pend = kq_thunks(p + 1) if p + 1 < NP else []
                if p == 0:
                    pend = [(lambda tt=tt: v_group(tt)) for tt in range(4, NKT)] + pend
                # ---- attention for this head pair ----
                for qb in range(NQB):
                    nkv = 4 * (qb + 1)   # kv tiles for this q block
                    y_ps = [ps.tile([128, 512], F32, tag=f"y{h}", bufs=1, name=f"yps{h}") for h in range(2)]
                    for g0 in range(0, nkv, G):
                        s_ps = ps.tile([128, 2 * G * 512], F32, tag="s", bufs=2)
                        for h in range(2):
                            hsl = slice(64 * h, 64 * (h + 1))
                            for j in range(G):
                                kt = g0 + j
                                mm = nc.tensor.matmul(
                                    s_ps[:, 512 * (G * h + j):512 * (G * h + j + 1)],
                                    kT_sb[p][hsl, 128 * kt:128 * (kt + 1)],
                                    qT_sb[p][hsl, QB * qb:QB * (qb + 1)],
                                    start=True, stop=True,
                                )
                                att_marker.setdefault(p, mm)
                                att_last = mm
                        p_sb = att.tile([128, 2 * G * 512], BF16, tag="p", bufs=4)
                        nc.scalar.activation(
                            out=p_sb[:],
                            in_=s_ps[:],
                            func=mybir.ActivationFunctionType.Exp,
                            scale=float(D) ** -0.5,
                        )
                        for h in range(2):
                            for j in range(G):
                                kt = g0 + j
                                r = kt - 4 * qb
                                if r >= 0:  # diagonal region: apply causal mask
                                    nc.vector.tensor_mul(
                                        out=p_sb[:, 512 * (G * h + j):512 * (G * h + j + 1)],
                                        in0=p_sb[:, 512 * (G * h + j):512 * (G * h + j + 1)],
                                        in1=msk_sb[:, QB * r:QB * (r + 1)],
                                    )
                        for h in range(2):
                            for j in range(G):
                                kt = g0 + j
                                v3 = v_sb[kt].rearrange("p (g e) -> p g e", g=HL)
                                nc.tensor.matmul(
                                    y_ps[h][0:65, :],
                                    v3[:, 2 * p + h, :],
                                    p_sb[:, 512 * (G * h + j):512 * (G * h + j + 1)],
                                    start=(kt == 0), stop=(kt == nkv - 1),
                                )
                        if pend and (p == 0 or qb >= 2):
                            if p == 0 or g0 % 2 == 1 or len(pend) > (NQB - qb) * 4:
                                pend.pop(0)()
                    # evacuate y (unnormalized) + packed sums
                    qsl = slice(QB * qb, QB * (qb + 1))
                    nc.vector.tensor_copy(out=yT_sb[p][0:64, qsl], in_=y_ps[0][0:64, :])
                    ytmp = att.tile([64, 512], BF16, tag="ytmp", bufs=2)
                    nc.vector.tensor_copy(out=ytmp[:], in_=y_ps[1][0:64, :])
                    nc.sync.dma_start(out=yT_sb[p][64:128, qsl], in_=ytmp[:])
                    for h in range(2):
                        stg = att.tile([65, 512], F32, tag=f"sumstg{h}", bufs=2, name=f"stg{h}")
                        nc.vector.tensor_copy(out=stg[64:65, :], in_=y_ps[h][64:65, :])
                        u = 8 * p + 2 * qb + h
                        nc.sync.dma_start(out=sums_pack[4 * u:4 * (u + 1), :], in_=stg[64:65, :])
                    # per-qb normalize (recip over the pair's 32 rows is idempotent)
                    prows = slice(32 * p, 32 * (p + 1))
                    nc.vector.reciprocal(out=recip_pack[prows, :], in_=sums_pack[prows, :])
                    nc.vector.tensor_copy(out=recip_pbf[prows, :], in_=recip_pack[prows, :])
                    for h in range(2):
                        base = 32 * p + 8 * qb + 4 * h
                        nc.sync.dma_start(
                            out=rdram[2 * p + h:2 * p + h + 1, qsl],
                            in_=recip_pbf[base:base + 4, :],
                        )
                    rb = att.tile([128, QB], BF16, tag="rb", bufs=2)
                    nc.sync.dma_start(out=rb[0:64, :], in_=rdram[2 * p:2 * p + 1, qsl].to_broadcast([64, QB]))
                    nc.sync.dma_start(out=rb[64:128, :], in_=rdram[2 * p + 1:2 * p + 2, qsl].to_broadcast([64, QB]))
                    nc.gpsimd.tensor_mul(out=yT_sb[p][:, qsl], in0=yT_sb[p][:, qsl], in1=rb[:])

                while pend:
                    pend.pop(0)()
                nc.sync.dma_start(out=cc_in[p][:], in_=yT_sb[p][:])
                nc.gpsimd.collective_compute(
                    "AllGather",
                    mybir.AluOpType.bypass,
                    replica_groups=[[0, 1], [2, 3], [4, 5], [6, 7]],
                    ins=[cc_in[p][:].opt()],
                    outs=[cc_out[p][:].opt()],
                )
            for p in range(NP):
                yg_dmas.append(nc.gpsimd.dma_start(out=yg_sb[p][:], in_=cc_out[p][0:128, :]))
                yg_dmas.append(nc.gpsimd.dma_start(out=yg_sb[NP + p][:], in_=cc_out[p][128:256, :]))
            for dma in yg_dmas:
                add_dep_helper(dma.ins, att_marker[NP - 1].ins, sync=True,
                               reason="defer yg readback past attention start of last pair")
            # ---- projection pass A: pairs 0..2 (runs as filler inside last pair's attention) ----
            o_part = [ypool.tile([128, 512], BF16, name=f"opart{tt}", tag="opart", bufs=NKT)
                      for tt in range(NKT)]
            chunksA = [(r, p) for r in range(2) for p in range(NP - 1)]

            def projA(tt):
                o_ps = ps.tile([128, 512], F32, tag="qkv", bufs=2, name="ops")
                first = None
                for ci, (r, p) in enumerate(chunksA):
                    mm = nc.tensor.matmul(
                        o_ps[:],
                        yg_sb[r * NP + p][:, 128 * tt:128 * (tt + 1)],
                        wp_sb[4 * r + p][:],
                        start=(ci == 0), stop=(ci == len(chunksA) - 1),
                    )
                    first = first or mm
                nc.vector.tensor_copy(out=o_part[tt][:], in_=o_ps[:])
                return first

            NA2 = 6
            for tt in range(NKT - NA2):
                projA(tt)
            a2_last = None
            for tt in range(NKT - NA2, NKT):
                first_mm = projA(tt)
                add_dep_helper(first_mm.ins, att_last.ins, sync=True,
                               reason="hold A2 groups for the last-AG window")
                a2_last = first_mm
            # ---- projection pass B: pair 3 chunks + partial, after last AG ----
            passb_first = None
            for tt in range(NKT):
                o_ps2 = ps.tile([128, 512], F32, tag="qkv", bufs=2)
                for ri, r in enumerate(range(2)):
                    mmb = nc.tensor.matmul(
                        o_ps2[:],
                        yg_sb[r * NP + NP - 1][:, 128 * tt:128 * (tt + 1)],
                        wp_sb[4 * r + NP - 1][:],
                        start=(ri == 0), stop=(ri == 1),
                    )
                    if passb_first is None:
                        passb_first = mmb
                        add_dep_helper(mmb.ins, a2_last.ins, sync=True,
                                       reason="pass-B after A2 filler")
                o_sb = att.tile([128, 512], F32, tag="osb", bufs=3)
                nc.vector.scalar_tensor_tensor(
                    out=o_sb[:], in0=o_ps2[:], scalar=1.0, in1=o_part[tt][:],
                    op0=mybir.AluOpType.mult, op1=mybir.AluOpType.add,
                )
                eng = nc.sync if tt % 2 == 0 else nc.scalar
                eng.dma_start(out=out.ap()[128 * tt:128 * (tt + 1), :], in_=o_sb[:])

    nc.compile()
    _NC_CACHE["nc"] = nc
    return nc


def make_in_maps(x, w_qkv, w_proj):
    bf = ml_dtypes.bfloat16
    # causal staircase masks for the 4 diagonal kv-tile offsets
    i = np.arange(128)[:, None]
    j = np.arange(QB)[None, :]
    msk = np.concatenate(
        [(r * 128 + i <= j).astype(bf) for r in range(4)], axis=1
    )  # [128, 2048]
    in_maps = []
    for core in range(8):
        beta, eta = core // 2, core % 2
        xT = np.ascontiguousarray(x[beta].T).astype(bf)
        wk = w_qkv[:, C + 512 * eta: C + 512 * (eta + 1)].astype(bf)
        wq = w_qkv[:, 512 * eta: 512 * (eta + 1)].astype(bf)
        wv = w_qkv[:, 2 * C + 512 * eta: 2 * C + 512 * (eta + 1)].astype(bf)
        wp = w_proj[:, 512 * eta: 512 * (eta + 1)].astype(bf)
        in_maps.append({"xT": xT, "wk": np.ascontiguousarray(wk),
                        "wq": np.ascontiguousarray(wq),
                        "wv": np.ascontiguousarray(wv),
                        "wp": np.ascontiguousarray(wp), "msk": msk})
    return in_maps


def assemble(results):
    out = np.empty((B, T, C), np.float32)
    for core in range(8):
        beta, eta = core // 2, core % 2
        out[beta, :, 512 * eta: 512 * (eta + 1)] = results[core]["out"]
    return out


def kernel(x, w_qkv, w_proj):
    x = np.asarray(x, np.float32)
    w_qkv = np.asarray(w_qkv, np.float32)
    w_proj = np.asarray(w_proj, np.float32)
    nc = build_nc()
    in_maps = make_in_maps(x, w_qkv, w_proj)
    res = run_bass_kernel_spmd(nc, in_maps, core_ids=list(range(8)))
    return assemble(res.results)


# revision 32
# speedup vs baseline: 1.0466x; 1.0406x over previous
"""Causal self-attention (b=4, t=2048, c=1024, h=16, d=64) on 8 TRN2 cores.

Sharding: core i -> batch i//2, head-half i%2 (8 heads), out-col-half i%2.
Per core: QKV (bf16) for its heads over its batch, flash-style causal
attention in transposed layout, pairwise AllGather of y^T, half of the
output projection columns. Output is exact-shape f32.
"""
import numpy as np
import ml_dtypes

import concourse.bass as bass
import concourse.mybir as mybir
import concourse.tile as tile
from concourse.tile_rust import add_dep_helper
from concourse import bacc
from concourse.bass_utils import run_bass_kernel_spmd

BF16 = mybir.dt.bfloat16
F32 = mybir.dt.float32

B, T, C = 4, 2048, 1024
H, D = 16, 64
HL = 8            # heads per core (local)
NP = HL // 2      # head pairs per core
QB = 512          # q block (free dim)
NQB = T // QB     # 4 q blocks
NKT = T // 128    # 16 kv tiles
NCH = C // 128    # 8 contraction chunks
G = 1             # kv tiles per score/exp group

_NC_CACHE = {}


def build_nc():
    if "nc" in _NC_CACHE:
        return _NC_CACHE["nc"]
    nc = bacc.Bacc("TRN2", target_bir_lowering=False, debug=False, num_devices=8)
    xT = nc.dram_tensor("xT", [C, T], BF16, kind="ExternalInput")
    wk = nc.dram_tensor("wk", [C, 512], BF16, kind="ExternalInput")
    wq = nc.dram_tensor("wq", [C, 512], BF16, kind="ExternalInput")
    wv = nc.dram_tensor("wv", [C, 512], BF16, kind="ExternalInput")
    wp = nc.dram_tensor("wp", [C, 512], BF16, kind="ExternalInput")
    msk = nc.dram_tensor("msk", [128, 4 * QB], BF16, kind="ExternalInput")
    out = nc.dram_tensor("out", [T, 512], F32, kind="ExternalOutput")

    with tile.TileContext(nc) as tc:
        with tc.tile_pool(name="w", bufs=1) as wpool, \
             tc.tile_pool(name="x", bufs=1) as xpool, \
             tc.tile_pool(name="kqv", bufs=1) as kqv, \
             tc.tile_pool(name="att", bufs=2) as att, \
             tc.tile_pool(name="y", bufs=1) as ypool, \
             tc.tile_pool(name="ps", bufs=1, space="PSUM") as ps, \
             tc.tile_pool(name="dram", bufs=1, space="DRAM") as dram:

            # ---- load inputs ----
            xT_sb = [xpool.tile([128, T], BF16, name=f"xT{ch}") for ch in range(NCH)]
            wk_sb = [wpool.tile([128, 512], BF16, name=f"wk{ch}") for ch in range(NCH)]
            wq_sb = [wpool.tile([128, 512], BF16, name=f"wq{ch}") for ch in range(NCH)]
            wv_sb = [wpool.tile([128, 512], BF16, name=f"wv{ch}") for ch in range(NCH)]
            wp_sb = [wpool.tile([128, 512], BF16, name=f"wp{ch}") for ch in range(NCH)]
            for ch in range(NCH):
                sl = slice(128 * ch, 128 * (ch + 1))
                eng = nc.sync if ch % 2 == 0 else nc.scalar
                other = nc.scalar if ch % 2 == 0 else nc.sync
                eng.dma_start(out=wk_sb[ch][:], in_=wk.ap()[sl, :])
                eng.dma_start(out=xT_sb[ch][:], in_=xT.ap()[sl, :])
                other.dma_start(out=wv_sb[ch][:], in_=wv.ap()[sl, :])
                other.dma_start(out=wq_sb[ch][:], in_=wq.ap()[sl, :])
            for ch in range(NCH):
                nc.scalar.dma_start(out=wp_sb[ch][:], in_=wp.ap()[128 * ch:128 * (ch + 1), :])
            msk_sb = wpool.tile([128, 4 * QB], BF16)
            nc.scalar.dma_start(out=msk_sb[:], in_=msk.ap()[:])

            # ---- V (token-major, with ones column per head) ----
            v_sb = [kqv.tile([128, HL * 65], BF16, name=f"v{tt}") for tt in range(NKT)]

            def v_group(tt):
                v3 = v_sb[tt].rearrange("p (g e) -> p g e", g=HL)
                v_ps = ps.tile([128, 512], F32, tag="qkv", bufs=2, name="vps")
                for ch in range(NCH):
                    nc.tensor.matmul(
                        v_ps[:],
                        xT_sb[ch][:, 128 * tt:128 * (tt + 1)],
                        wv_sb[ch][:],
                        start=(ch == 0), stop=(ch == NCH - 1),
                    )
                nc.vector.tensor_copy(
                    out=v3[:, :, 0:64],
                    in_=v_ps[:].rearrange("p (g e) -> p g e", g=HL),
                )
                nc.vector.memset(v3[:, :, 64:65], 1.0)

            # ---- K^T, Q^T (head-pair tiles [128, T]) ----
            kT_sb = [kqv.tile([128, T], BF16, name=f"kT{p}") for p in range(NP)]
            qT_sb = [kqv.tile([128, T], BF16, name=f"qT{p}") for p in range(NP)]
            yT_sb = [ypool.tile([128, T], BF16, name=f"yT{p}", tag="yt", bufs=NP) for p in range(NP)]
            sums_pack = ypool.tile([128, 128], F32)
            recip_pack = ypool.tile([128, 128], F32)
            recip_pbf = ypool.tile([128, 128], BF16)

            rdram = dram.tile([HL, T], BF16)
            cc_in = [dram.tile([128, T], BF16, name=f"ccin{p}") for p in range(NP)]
            cc_out = [dram.tile([256, T], BF16, name=f"ccout{p}") for p in range(NP)]
            cc_in3 = [dram.tile([128, T // 2], BF16, name=f"ccin3h{h}") for h in range(2)]
            cc_out3 = [dram.tile([256, T // 2], BF16, name=f"ccout3h{h}") for h in range(2)]
            yg_sb = [ypool.tile([128, T], BF16, name=f"yg{j}", tag="yg", bufs=2 * NP) for j in range(2 * NP)]

            def kq_group(p, w_sb, dst, nb, evac_eng=None):
                kq_ps = ps.tile([128, 512], F32, tag="qkv", bufs=2, name="kqps")
                for ch in range(NCH):
                    nc.tensor.matmul(
                        kq_ps[:],
                        w_sb[ch][:, 128 * p:128 * (p + 1)],
                        xT_sb[ch][:, 512 * nb:512 * (nb + 1)],
                        start=(ch == 0), stop=(ch == NCH - 1),
                    )
                if evac_eng == "scalar":
                    nc.scalar.copy(out=dst[p][:, 512 * nb:512 * (nb + 1)], in_=kq_ps[:])
                else:
                    nc.vector.tensor_copy(out=dst[p][:, 512 * nb:512 * (nb + 1)], in_=kq_ps[:])

            def kq_thunks(p, evac_eng=None):
                return [
                    (lambda w_sb=w_sb, dst=dst, nb=nb: kq_group(p, w_sb, dst, nb, evac_eng))
                    for (w_sb, dst) in ((wk_sb, kT_sb), (wq_sb, qT_sb))
                    for nb in range(T // 512)
                ]

            att_marker = {}
            yg_dmas = []
            for t in kq_thunks(0, evac_eng="scalar"):
                t()
            for tt in range(4):
                v_group(tt)
            for p in range(NP):
                pend = kq_thunks(p + 1) if p + 1 < NP else []
                if p == 0:
                    pend = [(lambda tt=tt: v_group(tt)) for tt in range(4, NKT)] + pend
                # ---- attention for this head pair ----
                for qb in range(NQB):
                    nkv = 4 * (qb + 1)   # kv tiles for this q block
                    y_ps = [ps.tile([128, 512], F32, tag=f"y{h}", bufs=1, name=f"yps{h}") for h in range(2)]
                    for g0 in range(0, nkv, G):
                        s_ps = ps.tile([128, 2 * G * 512], F32, tag="s", bufs=2)
                        for h in range(2):
                            hsl = slice(64 * h, 64 * (h + 1))
                            for j in range(G):
                                kt = g0 + j
                                mm = nc.tensor.matmul(
                                    s_ps[:, 512 * (G * h + j):512 * (G * h + j + 1)],
                                    kT_sb[p][hsl, 128 * kt:128 * (kt + 1)],
                                    qT_sb[p][hsl, QB * qb:QB * (qb + 1)],
                                    start=True, stop=True,
                                )
                                att_marker.setdefault(p, mm)
                                att_last = mm
                        p_sb = att.tile([128, 2 * G * 512], BF16, tag="p", bufs=4)
                        nc.scalar.activation(
                            out=p_sb[:],
                            in_=s_ps[:],
                            func=mybir.ActivationFunctionType.Exp,
                            scale=float(D) ** -0.5,
                        )
                        for h in range(2):
                            for j in range(G):
                                kt = g0 + j
                                r = kt - 4 * qb
                                if r >= 0:  # diagonal region: apply causal mask
                                    nc.vector.tensor_mul(
                                        out=p_sb[:, 512 * (G * h + j):512 * (G * h + j + 1)],
                                        in0=p_sb[:, 512 * (G * h + j):512 * (G * h + j + 1)],
                                        in1=msk_sb[:, QB * r:QB * (r + 1)],
                                    )
                        for h in range(2):
                            for j in range(G):
                                kt = g0 + j
                                v3 = v_sb[kt].rearrange("p (g e) -> p g e", g=HL)
                                nc.tensor.matmul(
                                    y_ps[h][0:65, :],
                                    v3[:, 2 * p + h, :],
                                    p_sb[:, 512 * (G * h + j):512 * (G * h + j + 1)],
                                    start=(kt == 0), stop=(kt == nkv - 1),
                                )
                        if pend and (p == 0 or qb >= 2):
                            if p == 0 or g0 % 2 == 1 or len(pend) > (NQB - qb) * 4:
                                pend.pop(0)()
                    # evacuate y (unnormalized) + packed sums
                    qsl = slice(QB * qb, QB * (qb + 1))
                    nc.vector.tensor_copy(out=yT_sb[p][0:64, qsl], in_=y_ps[0][0:64, :])
                    ytmp = att.tile([64, 512], BF16, tag="ytmp", bufs=2)
                    nc.vector.tensor_copy(out=ytmp[:], in_=y_ps[1][0:64, :])
                    nc.sync.dma_start(out=yT_sb[p][64:128, qsl], in_=ytmp[:])
                    for h in range(2):
                        stg = att.tile([65, 512], F32, tag=f"sumstg{h}", bufs=2, name=f"stg{h}")
                        nc.vector.tensor_copy(out=stg[64:65, :], in_=y_ps[h][64:65, :])
                        u = 8 * p + 2 * qb + h
                        nc.sync.dma_start(out=sums_pack[4 * u:4 * (u + 1), :], in_=stg[64:65, :])
                    # per-qb normalize (recip over the pair's 32 rows is idempotent)
                    prows = slice(32 * p, 32 * (p + 1))
                    nc.vector.reciprocal(out=recip_pack[prows, :], in_=sums_pack[prows, :])
                    nc.vector.tensor_copy(out=recip_pbf[prows, :], in_=recip_pack[prows, :])
                    for h in range(2):
                        base = 32 * p + 8 * qb + 4 * h
                        nc.sync.dma_start(
                            out=rdram[2 * p + h:2 * p + h + 1, qsl],
                            in_=recip_pbf[base:base + 4, :],
                        )
                    rb = att.tile([128, QB], BF16, tag="rb", bufs=2)
                    nc.sync.dma_start(out=rb[0:64, :], in_=rdram[2 * p:2 * p + 1, qsl].to_broadcast([64, QB]))
                    nc.sync.dma_start(out=rb[64:128, :], in_=rdram[2 * p + 1:2 * p + 2, qsl].to_broadcast([64, QB]))
                    nc.gpsimd.tensor_mul(out=yT_sb[p][:, qsl], in0=yT_sb[p][:, qsl], in1=rb[:])
                    if p == NP - 1 and qb % 2 == 1:
                        hf = qb // 2
                        hslc = slice((T // 2) * hf, (T // 2) * (hf + 1))
                        nc.sync.dma_start(out=cc_in3[hf][:], in_=yT_sb[p][:, hslc])
                        nc.gpsimd.collective_compute(
                            "AllGather",
                            mybir.AluOpType.bypass,
                            replica_groups=[[0, 1], [2, 3], [4, 5], [6, 7]],
                            ins=[cc_in3[hf][:].opt()],
                            outs=[cc_out3[hf][:].opt()],
                        )

                while pend:
                    pend.pop(0)()
                if p < NP - 1:
                    nc.sync.dma_start(out=cc_in[p][:], in_=yT_sb[p][:])
                    nc.gpsimd.collective_compute(
                        "AllGather",
                        mybir.AluOpType.bypass,
                        replica_groups=[[0, 1], [2, 3], [4, 5], [6, 7]],
                        ins=[cc_in[p][:].opt()],
                        outs=[cc_out[p][:].opt()],
                    )
            for p in range(NP - 1):
                yg_dmas.append(nc.gpsimd.dma_start(out=yg_sb[p][:], in_=cc_out[p][0:128, :]))
                yg_dmas.append(nc.gpsimd.dma_start(out=yg_sb[NP + p][:], in_=cc_out[p][128:256, :]))
            for hf in range(2):
                hslc = slice((T // 2) * hf, (T // 2) * (hf + 1))
                yg_dmas.append(nc.gpsimd.dma_start(out=yg_sb[NP - 1][:, hslc], in_=cc_out3[hf][0:128, :]))
                yg_dmas.append(nc.gpsimd.dma_start(out=yg_sb[2 * NP - 1][:, hslc], in_=cc_out3[hf][128:256, :]))
            for dma in yg_dmas:
                add_dep_helper(dma.ins, att_marker[NP - 1].ins, sync=True,
                               reason="defer yg readback past attention start of last pair")
            # ---- projection pass A: pairs 0..2 (runs as filler inside last pair's attention) ----
            o_part = [ypool.tile([128, 512], BF16, name=f"opart{tt}", tag="opart", bufs=NKT)
                      for tt in range(NKT)]
            chunksA = [(r, p) for r in range(2) for p in range(NP - 1)]

            def projA(tt):
                o_ps = ps.tile([128, 512], F32, tag="qkv", bufs=2, name="ops")
                first = None
                for ci, (r, p) in enumerate(chunksA):
                    mm = nc.tensor.matmul(
                        o_ps[:],
                        yg_sb[r * NP + p][:, 128 * tt:128 * (tt + 1)],
                        wp_sb[4 * r + p][:],
                        start=(ci == 0), stop=(ci == len(chunksA) - 1),
                    )
                    first = first or mm
                nc.vector.tensor_copy(out=o_part[tt][:], in_=o_ps[:])
                return first

            NA2 = 6
            for tt in range(NKT - NA2):
                projA(tt)
            a2_last = None
            for tt in range(NKT - NA2, NKT):
                first_mm = projA(tt)
                add_dep_helper(first_mm.ins, att_last.ins, sync=True,
                               reason="hold A2 groups for the last-AG window")
                a2_last = first_mm
            # ---- projection pass B: pair 3 chunks + partial, after last AG ----
            passb_first = None
            for tt in range(NKT):
                o_ps2 = ps.tile([128, 512], F32, tag="qkv", bufs=2)
                for ri, r in enumerate(range(2)):
                    mmb = nc.tensor.matmul(
                        o_ps2[:],
                        yg_sb[r * NP + NP - 1][:, 128 * tt:128 * (tt + 1)],
                        wp_sb[4 * r + NP - 1][:],
                        start=(ri == 0), stop=(ri == 1),
                    )
                    if passb_first is None:
                        passb_first = mmb
                        add_dep_helper(mmb.ins, a2_last.ins, sync=True,
                                       reason="pass-B after A2 filler")
                o_sb = att.tile([128, 512], F32, tag="osb", bufs=3)
                nc.vector.scalar_tensor_tensor(
                    out=o_sb[:], in0=o_ps2[:], scalar=1.0, in1=o_part[tt][:],
                    op0=mybir.AluOpType.mult, op1=mybir.AluOpType.add,
                )
                eng = nc.sync if tt % 2 == 0 else nc.scalar
                eng.dma_start(out=out.ap()[128 * tt:128 * (tt + 1), :], in_=o_sb[:])

    nc.compile()
    _NC_CACHE["nc"] = nc
    return nc


def make_in_maps(x, w_qkv, w_proj):
    bf = ml_dtypes.bfloat16
    # causal staircase masks for the 4 diagonal kv-tile offsets
    i = np.arange(128)[:, None]
    j = np.arange(QB)[None, :]
    msk = np.concatenate(
        [(r * 128 + i <= j).astype(bf) for r in range(4)], axis=1
    )  # [128, 2048]
    in_maps = []
    for core in range(8):
        beta, eta = core // 2, core % 2
        xT = np.ascontiguousarray(x[beta].T).astype(bf)
        wk = w_qkv[:, C + 512 * eta: C + 512 * (eta + 1)].astype(bf)
        wq = w_qkv[:, 512 * eta: 512 * (eta + 1)].astype(bf)
        wv = w_qkv[:, 2 * C + 512 * eta: 2 * C + 512 * (eta + 1)].astype(bf)
        wp = w_proj[:, 512 * eta: 512 * (eta + 1)].astype(bf)
        in_maps.append({"xT": xT, "wk": np.ascontiguousarray(wk),
                        "wq": np.ascontiguousarray(wq),
                        "wv": np.ascontiguousarray(wv),
                        "wp": np.ascontiguousarray(wp), "msk": msk})
    return in_maps


def assemble(results):
    out = np.empty((B, T, C), np.float32)
    for core in range(8):
        beta, eta = core // 2, core % 2
        out[beta, :, 512 * eta: 512 * (eta + 1)] = results[core]["out"]
    return out


def kernel(x, w_qkv, w_proj):
    x = np.asarray(x, np.float32)
    w_qkv = np.asarray(w_qkv, np.float32)
    w_proj = np.asarray(w_proj, np.float32)
    nc = build_nc()
    in_maps = make_in_maps(x, w_qkv, w_proj)
    res = run_bass_kernel_spmd(nc, in_maps, core_ids=list(range(8)))
    return assemble(res.results)
